# revision 14
# baseline (speedup 1.0000x reference)
"""Distributed GQA attention kernel for 8 TRN2 NeuronCores.

Problem: B=1, S=2048, D=4096, H=32 q-heads, KV=8 kv-heads, HD=128.
  q = rope(x@wq.T), k = rope(x@wk.T), v = x@wv.T
  out = softmax(causal(q@k.T/sqrt(HD))) @ v @ wo.T

Sharding: tensor-parallel over heads. Core c owns q-heads 4c..4c+3 and
kv-head c. Device-side per core:
  phase 1: QT/KT (rope'd, [hd, s] layout) + VT projections; rope runs
           off the PE (DVE muls + partition-swap DMA + DVE add); V
           tiles ([t, hd]) via DMA transpose.
  phase 2: causal attention in head PAIRS so the two M=1 rowsum
           matmuls pack into one PE slot via column tiling (psum
           partitions 0/64); softmax denominators broadcast on GpSimd;
           AllGather + out-proj scheduled so the final gather is
           covered by ~80us of deferred out-proj matmuls.
Host side: layout prep (transposes, bf16 cast, sign-folded rope
tables) + final concat/transpose of the 8 out.T slices.
"""

import math
import numpy as np
import ml_dtypes

BF = ml_dtypes.bfloat16

B, S, D = 1, 2048, 4096
H, KV, HD = 32, 8, 128
NCORES = 8
HL = H // NCORES            # 4 local q heads
QW = HL * HD                # 512 local q width
SC = 512                    # s-chunk width
NSC = S // SC               # 4 s-chunks
KD = 32                     # d-dim k-tiles (4096/128)
NT = S // 128               # 16 t-tiles
SCALE = 1.0 / math.sqrt(HD)
NEG = -30000.0

USE_GPSIMD_BC = True        # broadcast softmax denom on GpSimd (else PE)


def _build_nc():
    import concourse.bass as bass
    import concourse.mybir as mybir
    from concourse import bacc, tile

    dt = mybir.dt
    nc = bacc.Bacc()

    xt_d = nc.declare_dram_parameter("xt", [D, S], dt.bfloat16, isOutput=False)
    wqt_d = nc.declare_dram_parameter("wqt", [D, QW], dt.bfloat16, isOutput=False)
    wkt_d = nc.declare_dram_parameter("wkt", [D, HD], dt.bfloat16, isOutput=False)
    wvt_d = nc.declare_dram_parameter("wvt", [D, HD], dt.bfloat16, isOutput=False)
    wot_d = nc.declare_dram_parameter("wot", [D, QW], dt.bfloat16, isOutput=False)
    cosd_d = nc.declare_dram_parameter("cosd", [HD, S], dt.bfloat16, isOutput=False)
    sind_d = nc.declare_dram_parameter("sind", [HD, S], dt.bfloat16, isOutput=False)
    dmask_d = nc.declare_dram_parameter("dmask", [128, 128], dt.float32, isOutput=False)
    onesc_d = nc.declare_dram_parameter("onesc", [128, 1], dt.bfloat16, isOutput=False)
    onesr_d = nc.declare_dram_parameter("onesr", [1, 128], dt.bfloat16, isOutput=False)
    out_d = nc.declare_dram_parameter("out_t", [QW, S], dt.float32, isOutput=True)

    with tile.TileContext(nc) as tc:
        with (
            tc.tile_pool(name="const", bufs=1) as cpool,
            tc.tile_pool(name="qkv", bufs=1) as qkvpool,
            tc.tile_pool(name="att", bufs=1) as attpool,
            tc.tile_pool(name="dram", bufs=1, space="DRAM") as dpool,
        ):
            # ---- persistent activations ----
            qt = [qkvpool.tile([HD, S], dt.bfloat16, name=f"qt{h}", tag=f"qt{h}")
                  for h in range(HL)]
            kt = qkvpool.tile([HD, S], dt.bfloat16)
            vv = qkvpool.tile([128, NT, HD], dt.bfloat16)   # [t_part, ti, hd]
            att = [attpool.tile([HD, S], dt.bfloat16, name=f"att{h}", tag=f"att{h}")
                   for h in range(HL)]

            xt_r = xt_d[:, :].rearrange("(k p) s -> p k s", p=128)

            # small resident constants (emitted after the first x/wq pieces
            # below so those DMAs get queue-head positions)
            cosd = cpool.tile([HD, S], dt.bfloat16)
            sind = cpool.tile([HD, S], dt.bfloat16)
            dmask = cpool.tile([128, 128], dt.float32)
            onesc = cpool.tile([128, 1], dt.bfloat16)
            onesr = cpool.tile([1, 128], dt.bfloat16)

            # ================= phase 1: projections + rope =================
            with (
                tc.tile_pool(name="w1", bufs=1) as wpool,
                tc.tile_pool(name="xc", bufs=2) as xpool,
                tc.tile_pool(name="p1", bufs=6, space="PSUM") as pp1,
                tc.tile_pool(name="rtmp", bufs=3) as rtpool,
            ):
                wqt = wpool.tile([128, KD, QW], dt.bfloat16)
                wkt = wpool.tile([128, KD, HD], dt.bfloat16)
                wvt = wpool.tile([128, KD, HD], dt.bfloat16)
                vt = wpool.tile([HD, S], dt.bfloat16)
                wqt_r = wqt_d[:, :].rearrange("(k p) n -> p k n", p=128)
                wkt_r = wkt_d[:, :].rearrange("(k p) n -> p k n", p=128)
                wvt_r = wvt_d[:, :].rearrange("(k p) n -> p k n", p=128)
                # interleave the first x-chunk, wq pieces, rope tables and
                # k/v weights so everything lands just before its consumer:
                # first matmul ~2us, rope tables by ~10us (psum recycling
                # depends on the rope muls), wkt by ~35us, wvt by ~43us
                xc0 = xpool.tile([128, KD, SC], dt.bfloat16, tag="xc")

                def xq_piece(lo, hi):
                    ksl = slice(lo, hi)
                    nc.sync.dma_start(xc0[:, ksl, :], xt_r[:, ksl, 0:SC])
                    nc.sync.dma_start(wqt[:, ksl, :], wqt_r[:, ksl, :])

                xq_piece(0, 1)
                xq_piece(1, 2)
                nc.sync.dma_start(cosd[:], cosd_d[:, :])
                nc.sync.dma_start(sind[:], sind_d[:, :])
                xq_piece(2, 3)
                xq_piece(3, 4)
                nc.sync.dma_start(dmask[:], dmask_d[:, :])
                nc.sync.dma_start(onesc[:], onesc_d[:, :])
                xq_piece(4, 6)
                nc.sync.dma_start(wkt[:, 0:8, :], wkt_r[:, 0:8, :])
                xq_piece(6, 8)
                nc.sync.dma_start(wkt[:, 8:16, :], wkt_r[:, 8:16, :])
                xq_piece(8, 12)
                nc.sync.dma_start(wkt[:, 16:32, :], wkt_r[:, 16:32, :])
                xq_piece(12, 16)
                nc.sync.dma_start(wvt[:, 0:16, :], wvt_r[:, 0:16, :])
                xq_piece(16, 20)
                nc.sync.dma_start(wvt[:, 16:32, :], wvt_r[:, 16:32, :])
                xq_piece(20, 24)
                xq_piece(24, 28)
                xq_piece(28, 32)
                nc.sync.dma_start(onesr[:], onesr_d[:, :])
                # warm up the ACT exp table load before attention needs it
                warm = cpool.tile([1, 1], dt.float32)
                nc.scalar.activation(warm[:], dmask[0:1, 0:1],
                                     mybir.ActivationFunctionType.Exp)

                for sc in range(NSC):
                    ssl = slice(sc * SC, (sc + 1) * SC)
                    if sc == 0:
                        xc = xc0
                    else:
                        xc = xpool.tile([128, KD, SC], dt.bfloat16, tag="xc")
                        for kg in range(4):
                            ksl = slice(kg * 8, (kg + 1) * 8)
                            nc.sync.dma_start(xc[:, ksl, :], xt_r[:, ksl, ssl])

                    # 4 Q heads (rope), K (rope), V (plain) — all [hd, s]
                    for hi in range(HL + 2):
                        ps = pp1.tile([128, SC], dt.float32)
                        for k in range(KD):
                            if hi < HL:
                                lhs = wqt[:, k, hi * HD:(hi + 1) * HD]
                            elif hi == HL:
                                lhs = wkt[:, k, :]
                            else:
                                lhs = wvt[:, k, :]
                            nc.tensor.matmul(ps[:], lhs, xc[:, k, :],
                                             start=(k == 0), stop=(k == KD - 1))
                        if hi == HL + 1:
                            nc.vector.tensor_copy(vt[:, ssl], ps[:])
                            continue
                        # rope off-PE. q/k head rows are de-interleaved
                        # host-side (real parts rows 0-63, imag rows 64-127;
                        # scores are permutation-invariant over hd), so the
                        # rotate-half is two contiguous half-tile DMAs:
                        #   out = t*cos2 + swap_halves(t*sin2)
                        # with sin2 sign-folded (+s top half, -s bottom).
                        qc = rtpool.tile([128, SC], dt.bfloat16, tag="ropeqc")
                        qs = rtpool.tile([128, SC], dt.bfloat16, tag="ropeqs")
                        qw = rtpool.tile([128, SC], dt.bfloat16, tag="ropeqw")
                        nc.vector.tensor_mul(qc[:], ps[:], cosd[:, ssl])
                        nc.vector.tensor_mul(qs[:], ps[:], sind[:, ssl])
                        nc.sync.dma_start(qw[0:64, :], qs[64:128, :])
                        nc.sync.dma_start(qw[64:128, :], qs[0:64, :])
                        dst = qt[hi] if hi < HL else kt
                        nc.vector.tensor_add(dst[:, ssl], qc[:], qw[:])

                    # V tiles in [t, hd] layout via DMA transpose
                    for vtile in range(4):
                        ti = sc * 4 + vtile
                        nc.sync.dma_start_transpose(
                            vv[:, ti, :], vt[:, ti * 128:(ti + 1) * 128])

            # ============ phase 2+3: attention, allgather, out-proj ============
            with (
                tc.tile_pool(name="wo", bufs=1) as wopool,
                tc.tile_pool(name="agc", bufs=5) as agpool,
                tc.tile_pool(name="st", bufs=3, space="PSUM") as stpool,
                tc.tile_pool(name="pv", bufs=3, space="PSUM") as pvpool,
                tc.tile_pool(name="rs", bufs=1, space="PSUM") as rspool,
                tc.tile_pool(name="p3", bufs=1, space="PSUM") as pp3,
                tc.tile_pool(name="pt", bufs=8) as ptpool,
                tc.tile_pool(name="ep", bufs=3) as eppool,
                tc.tile_pool(name="ep1", bufs=1) as ep1pool,
                tc.tile_pool(name="o3", bufs=4) as opool,
            ):
                wot = wopool.tile([128, KD, QW], dt.bfloat16)
                nc.sync.dma_start(
                    wot[:], wot_d[:, :].rearrange("(k p) n -> p k n", p=128))

                # zero-dependency dummy gather (uninitialized data, result
                # unused) to absorb first-collective setup during phase 1
                cw_in = dpool.tile([128, 16], dt.bfloat16)
                cw_out = dpool.tile([NCORES * 128, 16], dt.bfloat16,
                                    addr_space="Shared")
                nc.gpsimd.collective_compute(
                    "AllGather",
                    mybir.AluOpType.bypass,
                    replica_groups=[list(range(NCORES))],
                    ins=[cw_in.opt()],
                    outs=[cw_out.opt()],
                )

                def epilogue(sc, h, pv, rs_row, shifted):
                    # reciprocal of the rowsums, broadcast across partitions,
                    # normalize on DVE. The custom-DVE reciprocal only works
                    # at base partition 0 (HW reads partition 0 whatever the
                    # AP says), so the partition-64 row of the col-packed
                    # pair is first moved down via regular-DVE copy + DMA.
                    ssl = slice(sc * SC, (sc + 1) * SC)
                    if shifted:
                        cp = ep1pool.tile([128, SC], dt.float32, tag="cp64")
                        nc.vector.tensor_copy(cp[64:65, :], rs_row)
                        row = eppool.tile([1, SC], dt.float32, tag="row64")
                        nc.sync.dma_start(row[:], cp[64:65, :])
                        rs_row = row[:]
                    rec = eppool.tile([1, SC], dt.float32, tag="rec")
                    nc.vector.reciprocal_approx_fast(rec[:], rs_row)
                    bcs = eppool.tile([128, SC], dt.float32, tag="bcs")
                    if USE_GPSIMD_BC:
                        nc.gpsimd.partition_broadcast(bcs[:], rec[:])
                    else:
                        nc.sync.dma_start(
                            bcs[:], rec[0:1, :].partition_broadcast(128))
                    nc.vector.tensor_mul(att[h][:, ssl], pv[:], bcs[:])

                def allgather_heads(sc, heads):
                    # gather this core's att rows for `heads`; out block r
                    # covers global i-tiles {4r+h for h in heads}
                    ssl = slice(sc * SC, (sc + 1) * SC)
                    nh = len(heads)
                    sfx = f"{sc}h{heads[0]}"
                    ag_in = dpool.tile([nh * HD, SC], dt.bfloat16,
                                       name=f"agi{sfx}", tag=f"agi{sfx}")
                    ag_out = dpool.tile([NCORES * nh * HD, SC], dt.bfloat16,
                                        name=f"ago{sfx}", tag=f"ago{sfx}",
                                        addr_space="Shared")
                    for i, h in enumerate(heads):
                        nc.sync.dma_start(ag_in[i * HD:(i + 1) * HD, :],
                                          att[h][:, ssl])
                    nc.gpsimd.collective_compute(
                        "AllGather",
                        mybir.AluOpType.bypass,
                        replica_groups=[list(range(NCORES))],
                        ins=[ag_in.opt()],
                        outs=[ag_out.opt()],
                    )
                    # preload the gathered chunk into SBUF right away so
                    # out-proj never waits on this DMA
                    ag_r = ag_out[:, :].rearrange("(m p) s -> p m s", p=128)
                    agc = agpool.tile([128, NCORES * nh, SC], dt.bfloat16,
                                      tag="agc")
                    nc.sync.dma_start(agc[:], ag_r[:, :, :])
                    return (agc, heads)

                def outproj_half(sc, piece, add_to=None):
                    # one gathered half (16 i-tiles); returns 4 sbuf tiles
                    ssl = slice(sc * SC, (sc + 1) * SC)
                    agc, heads = piece
                    nh = len(heads)
                    parts = []
                    for oc in range(4):
                        ps = pp3.tile([128, SC], dt.float32, tag="ps3")
                        for m in range(NCORES * nh):
                            kg = (m // nh) * HL + heads[m % nh]
                            nc.tensor.matmul(
                                ps[:], wot[:, kg, oc * 128:(oc + 1) * 128],
                                agc[:, m, :],
                                start=(m == 0), stop=(m == NCORES * nh - 1))
                        if add_to is None:
                            oa = opool.tile([128, SC], dt.float32, tag="oa")
                            nc.vector.tensor_copy(oa[:], ps[:])
                            parts.append(oa)
                        else:
                            ot = opool.tile([128, SC], dt.float32, tag="ot")
                            nc.vector.tensor_add(ot[:], ps[:], add_to[oc][:])
                            nc.sync.dma_start(
                                out_d[oc * 128:(oc + 1) * 128, ssl], ot[:])
                    return parts

                def outproj(sc, pieces):
                    parts = outproj_half(sc, pieces[0], add_to=None)
                    outproj_half(sc, pieces[1], add_to=parts)

                def scores_exp(sc, h, ti):
                    # emits scores matmul + diag mask + exp; returns (pt, v0)
                    d_off = ti * 128 - sc * SC
                    v0 = max(d_off, 0)
                    vsl = slice(v0, SC)
                    qcl = slice(sc * SC + v0, (sc + 1) * SC)
                    st = stpool.tile([128, SC], dt.float32, tag="st")
                    nc.tensor.matmul(st[:, vsl],
                                     kt[:, ti * 128:(ti + 1) * 128],
                                     qt[h][:, qcl], start=True, stop=True)
                    if d_off >= 0:
                        nc.vector.tensor_add(st[:, d_off:d_off + 128],
                                             st[:, d_off:d_off + 128],
                                             dmask[:])
                    pt = ptpool.tile([128, SC], dt.bfloat16, tag="pt")
                    nc.scalar.activation(pt[:, vsl], st[:, vsl],
                                         mybir.ActivationFunctionType.Exp,
                                         scale=SCALE)
                    return pt, v0

                # pair-interleaved attention: the two heads' M=1 rowsum
                # matmuls go to psum partitions 0/64 of one tile, so the PE
                # runs them concurrently via column tiling
                pending_ags = {}     # (sc, pairidx) -> gathered piece
                LOOKAHEAD = 2

                for sc in range(NSC):
                    n_t = sc * 4 + 4
                    last = (sc == NSC - 1)
                    pair_order = [(2, 3), (0, 1)] if last else [(0, 1), (2, 3)]
                    for pidx, (ha, hb) in enumerate(pair_order):
                        cache = {}
                        emitted = 0
                        pvs = {h: pvpool.tile([128, SC], dt.float32,
                                              name=f"pv{h}", tag="pv")
                               for h in (ha, hb)}
                        rs = rspool.tile([128, SC], dt.float32,
                                         name="rs", tag="rs")
                        rows = {ha: rs[0:1, :], hb: rs[64:65, :]}
                        for ti in range(n_t):
                            while emitted <= min(ti + LOOKAHEAD, n_t - 1):
                                for h in (ha, hb):
                                    cache[(h, emitted)] = scores_exp(sc, h, emitted)
                                emitted += 1
                            pta, v0a = cache.pop((ha, ti))
                            ptb, v0b = cache.pop((hb, ti))
                            fl = dict(start=(ti == 0), stop=(ti == n_t - 1))
                            nc.tensor.matmul(rs[0:1, v0a:], onesc[:],
                                             pta[:, v0a:], skip_group_check=True,
                                             **fl)
                            nc.tensor.matmul(rs[64:65, v0b:], onesc[:],
                                             ptb[:, v0b:], skip_group_check=True,
                                             **fl)
                            nc.tensor.matmul(pvs[ha][:, v0a:], vv[:, ti, :],
                                             pta[:, v0a:], **fl)
                            nc.tensor.matmul(pvs[hb][:, v0b:], vv[:, ti, :],
                                             ptb[:, v0b:], **fl)
                        epilogue(sc, ha, pvs[ha], rows[ha], shifted=False)
                        epilogue(sc, hb, pvs[hb], rows[hb], shifted=True)
                        pending_ags[(sc, pidx)] = allgather_heads(
                            sc, list(pair_order[pidx]))
                        # deferred out-proj emission as PE filler:
                        #   sc2 end -> outproj(0), sc3 between pairs ->
                        #   outproj(1); placed so each gather has ~25us+
                        #   of trigger->CC->sbuf-load latency covered
                        if sc == 2 and pidx == 1:
                            outproj(0, [pending_ags.pop((0, 0)),
                                        pending_ags.pop((0, 1))])
                        if sc == 3 and pidx == 0:
                            outproj(1, [pending_ags.pop((1, 0)),
                                        pending_ags.pop((1, 1))])
                # tail: outproj(2) + outproj(3a) cover the last AllGather
                outproj(2, [pending_ags.pop((2, 0)), pending_ags.pop((2, 1))])
                # chunk 3 pairs were processed in order (2,3),(0,1)
                parts3 = outproj_half(3, pending_ags.pop((3, 0)), add_to=None)
                outproj_half(3, pending_ags.pop((3, 1)), add_to=parts3)
    if not nc.is_finalized():
        nc.finalize()
    return nc


_CACHE = {}


def _get_nc():
    if "nc" not in _CACHE:
        _CACHE["nc"] = _build_nc()
    return _CACHE["nc"]


def _prep_in_maps(x, wq, wk, wv, wo, freqs_cos, freqs_sin):
    xt = np.ascontiguousarray(x.reshape(S, D).T).astype(BF)
    # rope rows are de-interleaved: real lanes -> rows 0-63, imag -> 64-127
    # (wq/wk output rows permuted to match; scores are invariant since q and
    # k share the permutation). sin is sign-folded: +s top half, -s bottom.
    ct = np.asarray(freqs_cos, np.float32).T   # [HD//2, S]
    st = np.asarray(freqs_sin, np.float32).T
    cosd = np.concatenate([ct, ct], axis=0).astype(BF)
    sind = np.concatenate([st, -st], axis=0).astype(BF)
    t_idx = np.arange(128)[:, None]
    s_idx = np.arange(128)[None, :]
    dmask = np.where(s_idx >= t_idx, 0.0, NEG).astype(np.float32)
    onesc = np.ones((128, 1), np.float32).astype(BF)
    onesr = np.ones((1, 128), np.float32).astype(BF)

    deint = np.concatenate([np.arange(0, HD, 2), np.arange(1, HD, 2)])
    wq = np.asarray(wq, np.float32).reshape(H, HD, D)[:, deint, :].reshape(H * HD, D)
    wk = np.asarray(wk, np.float32).reshape(KV, HD, D)[:, deint, :].reshape(KV * HD, D)
    wv = np.asarray(wv, np.float32)
    wo = np.asarray(wo, np.float32)

    in_maps = []
    for c in range(NCORES):
        qsl = slice(QW * c, QW * (c + 1))
        ksl = slice(HD * c, HD * (c + 1))
        in_maps.append({
            "xt": xt,
            "wqt": np.ascontiguousarray(wq[qsl].T).astype(BF),
            "wkt": np.ascontiguousarray(wk[ksl].T).astype(BF),
            "wvt": np.ascontiguousarray(wv[ksl].T).astype(BF),
            "wot": np.ascontiguousarray(wo[qsl].T).astype(BF),
            "cosd": cosd, "sind": sind,
            "dmask": dmask, "onesc": onesc, "onesr": onesr,
        })
    return in_maps


def run(inputs, trace=False):
    from concourse.bass_utils import run_bass_kernel_spmd
    nc = _get_nc()
    in_maps = _prep_in_maps(
        inputs["x"], inputs["wq"], inputs["wk"], inputs["wv"], inputs["wo"],
        inputs["freqs_cos"], inputs["freqs_sin"])
    res = run_bass_kernel_spmd(nc, in_maps, core_ids=list(range(NCORES)),
                               trace=trace)
    shards = [np.asarray(res.results[c]["out_t"], np.float32)
              for c in range(NCORES)]
    full = np.concatenate(shards, axis=0)          # [4096, 2048]
    out = np.ascontiguousarray(full.T)[None]       # [1, 2048, 4096]
    return out.astype(np.float32), res


def kernel(**inputs):
    out, _ = run(inputs, trace=False)
    return out


# revision 21
# speedup vs baseline: 1.0207x; 1.0207x over previous
"""Distributed GQA attention kernel for 8 TRN2 NeuronCores.

Problem: B=1, S=2048, D=4096, H=32 q-heads, KV=8 kv-heads, HD=128.
  q = rope(x@wq.T), k = rope(x@wk.T), v = x@wv.T
  out = softmax(causal(q@k.T/sqrt(HD))) @ v @ wo.T

Sharding: tensor-parallel over heads. Core c owns q-heads 4c..4c+3 and
kv-head c. Device-side per core:
  phase 1: QT/KT (rope'd, [hd, s] layout) + VT projections; rope runs
           off the PE (DVE muls + partition-swap DMA + DVE add); V
           tiles ([t, hd]) via DMA transpose.
  phase 2: causal attention in head PAIRS so the two M=1 rowsum
           matmuls pack into one PE slot via column tiling (psum
           partitions 0/64); softmax denominators broadcast on GpSimd;
           AllGather + out-proj scheduled so the final gather is
           covered by ~80us of deferred out-proj matmuls.
Host side: layout prep (transposes, bf16 cast, sign-folded rope
tables) + final concat/transpose of the 8 out.T slices.
"""

import math
import numpy as np
import ml_dtypes

BF = ml_dtypes.bfloat16

B, S, D = 1, 2048, 4096
H, KV, HD = 32, 8, 128
NCORES = 8
HL = H // NCORES            # 4 local q heads
QW = HL * HD                # 512 local q width
SC = 512                    # s-chunk width
NSC = S // SC               # 4 s-chunks
KD = 32                     # d-dim k-tiles (4096/128)
NT = S // 128               # 16 t-tiles
SCALE = 1.0 / math.sqrt(HD)
NEG = -30000.0

USE_GPSIMD_BC = True        # broadcast softmax denom on GpSimd (else PE)


def _build_nc():
    import concourse.bass as bass
    import concourse.mybir as mybir
    from concourse import bacc, tile

    dt = mybir.dt
    nc = bacc.Bacc()

    xt_d = nc.declare_dram_parameter("xt", [D, S], dt.bfloat16, isOutput=False)
    wqt_d = nc.declare_dram_parameter("wqt", [D, QW], dt.bfloat16, isOutput=False)
    wkt_d = nc.declare_dram_parameter("wkt", [D, HD], dt.bfloat16, isOutput=False)
    wvt_d = nc.declare_dram_parameter("wvt", [D, HD], dt.bfloat16, isOutput=False)
    wot_d = nc.declare_dram_parameter("wot", [D, QW], dt.bfloat16, isOutput=False)
    cosd_d = nc.declare_dram_parameter("cosd", [HD, S], dt.bfloat16, isOutput=False)
    sind_d = nc.declare_dram_parameter("sind", [HD, S], dt.bfloat16, isOutput=False)
    dmask_d = nc.declare_dram_parameter("dmask", [128, 128], dt.float32, isOutput=False)
    onesc_d = nc.declare_dram_parameter("onesc", [128, 1], dt.bfloat16, isOutput=False)
    onesr_d = nc.declare_dram_parameter("onesr", [1, 128], dt.bfloat16, isOutput=False)
    out_d = nc.declare_dram_parameter("out_t", [QW, S], dt.float32, isOutput=True)

    with tile.TileContext(nc) as tc:
        with (
            tc.tile_pool(name="const", bufs=1) as cpool,
            tc.tile_pool(name="qkv", bufs=1) as qkvpool,
            tc.tile_pool(name="att", bufs=1) as attpool,
            tc.tile_pool(name="dram", bufs=1, space="DRAM") as dpool,
        ):
            # ---- persistent activations ----
            qt = [qkvpool.tile([HD, S], dt.bfloat16, name=f"qt{h}", tag=f"qt{h}")
                  for h in range(HL)]
            kt = qkvpool.tile([HD, S], dt.bfloat16)
            vv = qkvpool.tile([128, NT, HD], dt.bfloat16)   # [t_part, ti, hd]
            att = [attpool.tile([HD, S], dt.bfloat16, name=f"att{h}", tag=f"att{h}")
                   for h in range(HL)]

            xt_r = xt_d[:, :].rearrange("(k p) s -> p k s", p=128)

            # small resident constants (emitted after the first x/wq pieces
            # below so those DMAs get queue-head positions)
            cosd = cpool.tile([HD, S], dt.bfloat16)
            sind = cpool.tile([HD, S], dt.bfloat16)
            dmask = cpool.tile([128, 128], dt.float32)
            onesc = cpool.tile([128, 1], dt.bfloat16)
            onesr = cpool.tile([1, 128], dt.bfloat16)

            # ================= phase 1: projections + rope =================
            with (
                tc.tile_pool(name="w1", bufs=1) as wpool,
                tc.tile_pool(name="xc", bufs=2) as xpool,
                tc.tile_pool(name="p1", bufs=6, space="PSUM") as pp1,
                tc.tile_pool(name="rtmp", bufs=3) as rtpool,
            ):
                wqt = wpool.tile([128, KD, QW], dt.bfloat16)
                wkt = wpool.tile([128, KD, HD], dt.bfloat16)
                wvt = wpool.tile([128, KD, HD], dt.bfloat16)
                vt = wpool.tile([HD, S], dt.bfloat16)
                wqt_r = wqt_d[:, :].rearrange("(k p) n -> p k n", p=128)
                wkt_r = wkt_d[:, :].rearrange("(k p) n -> p k n", p=128)
                wvt_r = wvt_d[:, :].rearrange("(k p) n -> p k n", p=128)
                # interleave the first x-chunk, wq pieces, rope tables and
                # k/v weights so everything lands just before its consumer:
                # first matmul ~2us, rope tables by ~10us (psum recycling
                # depends on the rope muls), wkt by ~35us, wvt by ~43us
                xc0 = xpool.tile([128, KD, SC], dt.bfloat16, tag="xc")

                def xq_piece(lo, hi):
                    ksl = slice(lo, hi)
                    nc.sync.dma_start(xc0[:, ksl, :], xt_r[:, ksl, 0:SC])
                    nc.sync.dma_start(wqt[:, ksl, :], wqt_r[:, ksl, :])

                # small consts first (rope tables gate psum recycling)
                nc.sync.dma_start(cosd[:], cosd_d[:, :])
                nc.sync.dma_start(sind[:], sind_d[:, :])
                nc.sync.dma_start(dmask[:], dmask_d[:, :])
                nc.sync.dma_start(onesc[:], onesc_d[:, :])
                nc.sync.dma_start(onesr[:], onesr_d[:, :])
                xq_piece(0, 1)
                xq_piece(1, 2)
                xq_piece(2, 4)
                xq_piece(4, 8)
                xq_piece(8, 12)
                nc.sync.dma_start(wkt[:, 0:16, :], wkt_r[:, 0:16, :])
                xq_piece(12, 16)
                nc.sync.dma_start(wkt[:, 16:32, :], wkt_r[:, 16:32, :])
                xq_piece(16, 20)
                nc.sync.dma_start(wvt[:, 0:16, :], wvt_r[:, 0:16, :])
                xq_piece(20, 24)
                nc.sync.dma_start(wvt[:, 16:32, :], wvt_r[:, 16:32, :])
                xq_piece(24, 28)
                xq_piece(28, 32)
                # warm up the ACT exp table load before attention needs it
                warm = cpool.tile([1, 1], dt.float32)
                nc.scalar.activation(warm[:], dmask[0:1, 0:1],
                                     mybir.ActivationFunctionType.Exp)

                xc_next = xc0
                nxt_pieces = [(0, 6), (6, 12), (12, 18), (18, 24),
                              (24, 29), (29, 32)]
                for sc in range(NSC):
                    ssl = slice(sc * SC, (sc + 1) * SC)
                    xc = xc_next
                    if sc + 1 < NSC:
                        # prefetch next chunk's x in pieces interleaved with
                        # this chunk's matmul targets (keeps the bulk queue
                        # fed well ahead of the consumers)
                        xc_next = xpool.tile([128, KD, SC], dt.bfloat16,
                                             tag="xc")
                        nssl = slice((sc + 1) * SC, (sc + 2) * SC)

                    # 4 Q heads (rope), K (rope), V (plain) — all [hd, s]
                    for hi in range(HL + 2):
                        if sc + 1 < NSC:
                            lo, hi2 = nxt_pieces[hi]
                            nc.sync.dma_start(xc_next[:, lo:hi2, :],
                                              xt_r[:, lo:hi2, nssl])
                        ps = pp1.tile([128, SC], dt.float32)
                        for k in range(KD):
                            if hi < HL:
                                lhs = wqt[:, k, hi * HD:(hi + 1) * HD]
                            elif hi == HL:
                                lhs = wkt[:, k, :]
                            else:
                                lhs = wvt[:, k, :]
                            nc.tensor.matmul(ps[:], lhs, xc[:, k, :],
                                             start=(k == 0), stop=(k == KD - 1))
                        if hi == HL + 1:
                            nc.vector.tensor_copy(vt[:, ssl], ps[:])
                            continue
                        # rope off-PE. q/k head rows are de-interleaved
                        # host-side (real parts rows 0-63, imag rows 64-127;
                        # scores are permutation-invariant over hd), so the
                        # rotate-half is two contiguous half-tile DMAs:
                        #   out = t*cos2 + swap_halves(t*sin2)
                        # with sin2 sign-folded (+s top half, -s bottom).
                        qc = rtpool.tile([128, SC], dt.bfloat16, tag="ropeqc")
                        qs = rtpool.tile([128, SC], dt.bfloat16, tag="ropeqs")
                        qw = rtpool.tile([128, SC], dt.bfloat16, tag="ropeqw")
                        nc.vector.tensor_mul(qc[:], ps[:], cosd[:, ssl])
                        nc.vector.tensor_mul(qs[:], ps[:], sind[:, ssl])
                        # compute-dependent DMAs go on the ACT engine's DGE
                        # queue: they'd head-of-line block the bulk loads on
                        # the sync queue while waiting for the DVE muls
                        nc.scalar.dma_start(qw[0:64, :], qs[64:128, :])
                        nc.scalar.dma_start(qw[64:128, :], qs[0:64, :])
                        dst = qt[hi] if hi < HL else kt
                        nc.vector.tensor_add(dst[:, ssl], qc[:], qw[:])

                    # V tiles in [t, hd] layout via DMA transpose
                    for vtile in range(4):
                        ti = sc * 4 + vtile
                        nc.scalar.dma_start_transpose(
                            vv[:, ti, :], vt[:, ti * 128:(ti + 1) * 128])

            # ============ phase 2+3: attention, allgather, out-proj ============
            with (
                tc.tile_pool(name="wo", bufs=1) as wopool,
                tc.tile_pool(name="agc", bufs=5) as agpool,
                tc.tile_pool(name="st", bufs=3, space="PSUM") as stpool,
                tc.tile_pool(name="pv", bufs=3, space="PSUM") as pvpool,
                tc.tile_pool(name="rs", bufs=1, space="PSUM") as rspool,
                tc.tile_pool(name="p3", bufs=1, space="PSUM") as pp3,
                tc.tile_pool(name="pt", bufs=8) as ptpool,
                tc.tile_pool(name="ep", bufs=3) as eppool,
                tc.tile_pool(name="ep1", bufs=1) as ep1pool,
                tc.tile_pool(name="o3", bufs=4) as opool,
            ):
                wot = wopool.tile([128, KD, QW], dt.bfloat16)
                nc.sync.dma_start(
                    wot[:], wot_d[:, :].rearrange("(k p) n -> p k n", p=128))

                # zero-dependency dummy gather (uninitialized data, result
                # unused) to absorb first-collective setup during phase 1
                cw_in = dpool.tile([128, 16], dt.bfloat16)
                cw_out = dpool.tile([NCORES * 128, 16], dt.bfloat16,
                                    addr_space="Shared")
                nc.gpsimd.collective_compute(
                    "AllGather",
                    mybir.AluOpType.bypass,
                    replica_groups=[list(range(NCORES))],
                    ins=[cw_in.opt()],
                    outs=[cw_out.opt()],
                )

                def epilogue_pair(sc, ha, hb, pvs, rs):
                    # Normalize attnT by 1/rowsum. pv tiles are copied out of
                    # PSUM immediately (bf16) so the banks recycle without
                    # waiting on the broadcast chain. The custom-DVE
                    # reciprocal only works at base partition 0 (HW reads
                    # partition 0 whatever the AP says), so the col-packed
                    # partition-64 row is moved down via regular-DVE copy +
                    # a gpsimd-queue DMA (won't block bulk loads).
                    ssl = slice(sc * SC, (sc + 1) * SC)
                    pvc = {}
                    for h in (ha, hb):
                        t = eppool.tile([128, SC], dt.bfloat16, tag="pvc")
                        nc.vector.tensor_copy(t[:], pvs[h][:])
                        pvc[h] = t
                    cp = ep1pool.tile([128, SC], dt.float32, tag="cp64")
                    nc.vector.tensor_copy(cp[64:65, :], rs[64:65, :])
                    rowb = eppool.tile([1, SC], dt.float32, tag="row64")
                    nc.gpsimd.dma_start(rowb[:], cp[64:65, :])
                    rows = {ha: rs[0:1, :], hb: rowb[:]}
                    for h in (ha, hb):
                        rec = eppool.tile([1, SC], dt.float32, tag="rec")
                        nc.vector.reciprocal_approx_fast(rec[:], rows[h])
                        recb = eppool.tile([1, SC], dt.bfloat16, tag="recb")
                        nc.vector.tensor_copy(recb[:], rec[:])
                        bcs = eppool.tile([128, SC], dt.bfloat16, tag="bcs")
                        nc.gpsimd.partition_broadcast(bcs[:], recb[:])
                        nc.vector.tensor_mul(att[h][:, ssl], pvc[h][:], bcs[:])

                def allgather_heads(sc, heads):
                    # gather this core's att rows for `heads`; out block r
                    # covers global i-tiles {4r+h for h in heads}
                    ssl = slice(sc * SC, (sc + 1) * SC)
                    nh = len(heads)
                    sfx = f"{sc}h{heads[0]}"
                    ag_in = dpool.tile([nh * HD, SC], dt.bfloat16,
                                       name=f"agi{sfx}", tag=f"agi{sfx}")
                    ag_out = dpool.tile([NCORES * nh * HD, SC], dt.bfloat16,
                                        name=f"ago{sfx}", tag=f"ago{sfx}",
                                        addr_space="Shared")
                    for i, h in enumerate(heads):
                        # gpsimd queue: depends on the att normalize, would
                        # head-of-line block bulk loads on the sync queue
                        nc.gpsimd.dma_start(ag_in[i * HD:(i + 1) * HD, :],
                                            att[h][:, ssl])
                    nc.gpsimd.collective_compute(
                        "AllGather",
                        mybir.AluOpType.bypass,
                        replica_groups=[list(range(NCORES))],
                        ins=[ag_in.opt()],
                        outs=[ag_out.opt()],
                    )
                    # preload the gathered chunk into SBUF right away so
                    # out-proj never waits on this DMA
                    ag_r = ag_out[:, :].rearrange("(m p) s -> p m s", p=128)
                    agc = agpool.tile([128, NCORES * nh, SC], dt.bfloat16,
                                      tag="agc")
                    nc.sync.dma_start(agc[:], ag_r[:, :, :])
                    return (agc, heads)

                def outproj_half(sc, piece, add_to=None):
                    # one gathered half (16 i-tiles); returns 4 sbuf tiles
                    ssl = slice(sc * SC, (sc + 1) * SC)
                    agc, heads = piece
                    nh = len(heads)
                    parts = []
                    for oc in range(4):
                        ps = pp3.tile([128, SC], dt.float32, tag="ps3")
                        for m in range(NCORES * nh):
                            kg = (m // nh) * HL + heads[m % nh]
                            nc.tensor.matmul(
                                ps[:], wot[:, kg, oc * 128:(oc + 1) * 128],
                                agc[:, m, :],
                                start=(m == 0), stop=(m == NCORES * nh - 1))
                        if add_to is None:
                            oa = opool.tile([128, SC], dt.float32, tag="oa")
                            nc.vector.tensor_copy(oa[:], ps[:])
                            parts.append(oa)
                        else:
                            ot = opool.tile([128, SC], dt.float32, tag="ot")
                            nc.vector.tensor_add(ot[:], ps[:], add_to[oc][:])
                            nc.sync.dma_start(
                                out_d[oc * 128:(oc + 1) * 128, ssl], ot[:])
                    return parts

                def outproj(sc, pieces):
                    parts = outproj_half(sc, pieces[0], add_to=None)
                    outproj_half(sc, pieces[1], add_to=parts)

                def scores_exp(sc, h, ti):
                    # emits scores matmul + diag mask + exp; returns (pt, v0)
                    d_off = ti * 128 - sc * SC
                    v0 = max(d_off, 0)
                    vsl = slice(v0, SC)
                    qcl = slice(sc * SC + v0, (sc + 1) * SC)
                    st = stpool.tile([128, SC], dt.float32, tag="st")
                    nc.tensor.matmul(st[:, vsl],
                                     kt[:, ti * 128:(ti + 1) * 128],
                                     qt[h][:, qcl], start=True, stop=True)
                    if d_off >= 0:
                        nc.vector.tensor_add(st[:, d_off:d_off + 128],
                                             st[:, d_off:d_off + 128],
                                             dmask[:])
                    pt = ptpool.tile([128, SC], dt.bfloat16, tag="pt")
                    nc.scalar.activation(pt[:, vsl], st[:, vsl],
                                         mybir.ActivationFunctionType.Exp,
                                         scale=SCALE)
                    return pt, v0

                # pair-interleaved attention: the two heads' M=1 rowsum
                # matmuls go to psum partitions 0/64 of one tile, so the PE
                # runs them concurrently via column tiling
                pending_ags = {}     # (sc, pairidx) -> gathered piece
                LOOKAHEAD = 2

                for sc in range(NSC):
                    n_t = sc * 4 + 4
                    last = (sc == NSC - 1)
                    pair_order = [(2, 3), (0, 1)] if last else [(0, 1), (2, 3)]
                    for pidx, (ha, hb) in enumerate(pair_order):
                        cache = {}
                        emitted = 0
                        pvs = {h: pvpool.tile([128, SC], dt.float32,
                                              name=f"pv{h}", tag="pv")
                               for h in (ha, hb)}
                        rs = rspool.tile([128, SC], dt.float32,
                                         name="rs", tag="rs")
                        for ti in range(n_t):
                            while emitted <= min(ti + LOOKAHEAD, n_t - 1):
                                for h in (ha, hb):
                                    cache[(h, emitted)] = scores_exp(sc, h, emitted)
                                emitted += 1
                            pta, v0a = cache.pop((ha, ti))
                            ptb, v0b = cache.pop((hb, ti))
                            fl = dict(start=(ti == 0), stop=(ti == n_t - 1))
                            nc.tensor.matmul(rs[0:1, v0a:], onesc[:],
                                             pta[:, v0a:], skip_group_check=True,
                                             **fl)
                            nc.tensor.matmul(rs[64:65, v0b:], onesc[:],
                                             ptb[:, v0b:], skip_group_check=True,
                                             **fl)
                            nc.tensor.matmul(pvs[ha][:, v0a:], vv[:, ti, :],
                                             pta[:, v0a:], **fl)
                            nc.tensor.matmul(pvs[hb][:, v0b:], vv[:, ti, :],
                                             ptb[:, v0b:], **fl)
                        epilogue_pair(sc, ha, hb, pvs, rs)
                        pending_ags[(sc, pidx)] = allgather_heads(
                            sc, list(pair_order[pidx]))
                        # deferred out-proj emission as PE filler:
                        #   sc2 end -> outproj(0), sc3 between pairs ->
                        #   outproj(1); placed so each gather has ~25us+
                        #   of trigger->CC->sbuf-load latency covered
                        if sc == 2 and pidx == 1:
                            outproj(0, [pending_ags.pop((0, 0)),
                                        pending_ags.pop((0, 1))])
                        if sc == 3 and pidx == 0:
                            outproj(1, [pending_ags.pop((1, 0)),
                                        pending_ags.pop((1, 1))])
                # tail: outproj(2) + outproj(3a) cover the last AllGather
                outproj(2, [pending_ags.pop((2, 0)), pending_ags.pop((2, 1))])
                # chunk 3 pairs were processed in order (2,3),(0,1)
                parts3 = outproj_half(3, pending_ags.pop((3, 0)), add_to=None)
                outproj_half(3, pending_ags.pop((3, 1)), add_to=parts3)
    if not nc.is_finalized():
        nc.finalize()
    return nc


_CACHE = {}


def _get_nc():
    if "nc" not in _CACHE:
        _CACHE["nc"] = _build_nc()
    return _CACHE["nc"]


def _prep_in_maps(x, wq, wk, wv, wo, freqs_cos, freqs_sin):
    xt = np.ascontiguousarray(x.reshape(S, D).T).astype(BF)
    # rope rows are de-interleaved: real lanes -> rows 0-63, imag -> 64-127
    # (wq/wk output rows permuted to match; scores are invariant since q and
    # k share the permutation). sin is sign-folded: +s top half, -s bottom.
    ct = np.asarray(freqs_cos, np.float32).T   # [HD//2, S]
    st = np.asarray(freqs_sin, np.float32).T
    cosd = np.concatenate([ct, ct], axis=0).astype(BF)
    sind = np.concatenate([st, -st], axis=0).astype(BF)
    t_idx = np.arange(128)[:, None]
    s_idx = np.arange(128)[None, :]
    dmask = np.where(s_idx >= t_idx, 0.0, NEG).astype(np.float32)
    onesc = np.ones((128, 1), np.float32).astype(BF)
    onesr = np.ones((1, 128), np.float32).astype(BF)

    deint = np.concatenate([np.arange(0, HD, 2), np.arange(1, HD, 2)])
    wq = np.asarray(wq, np.float32).reshape(H, HD, D)[:, deint, :].reshape(H * HD, D)
    wk = np.asarray(wk, np.float32).reshape(KV, HD, D)[:, deint, :].reshape(KV * HD, D)
    wv = np.asarray(wv, np.float32)
    wo = np.asarray(wo, np.float32)

    in_maps = []
    for c in range(NCORES):
        qsl = slice(QW * c, QW * (c + 1))
        ksl = slice(HD * c, HD * (c + 1))
        in_maps.append({
            "xt": xt,
            "wqt": np.ascontiguousarray(wq[qsl].T).astype(BF),
            "wkt": np.ascontiguousarray(wk[ksl].T).astype(BF),
            "wvt": np.ascontiguousarray(wv[ksl].T).astype(BF),
            "wot": np.ascontiguousarray(wo[qsl].T).astype(BF),
            "cosd": cosd, "sind": sind,
            "dmask": dmask, "onesc": onesc, "onesr": onesr,
        })
    return in_maps


def run(inputs, trace=False):
    from concourse.bass_utils import run_bass_kernel_spmd
    nc = _get_nc()
    in_maps = _prep_in_maps(
        inputs["x"], inputs["wq"], inputs["wk"], inputs["wv"], inputs["wo"],
        inputs["freqs_cos"], inputs["freqs_sin"])
    res = run_bass_kernel_spmd(nc, in_maps, core_ids=list(range(NCORES)),
                               trace=trace)
    shards = [np.asarray(res.results[c]["out_t"], np.float32)
              for c in range(NCORES)]
    full = np.concatenate(shards, axis=0)          # [4096, 2048]
    out = np.ascontiguousarray(full.T)[None]       # [1, 2048, 4096]
    return out.astype(np.float32), res


def kernel(**inputs):
    out, _ = run(inputs, trace=False)
    return out
